# revision 1
# baseline (speedup 1.0000x reference)
"""
Multi-head attention + residual + LayerNorm Trainium2 kernel (8 NeuronCores).

Problem (hardcoded shapes):
    hidden_states [2, 2048, 1024] f32, mask [2, 2048, 2048] int32,
    Wq/Wk/Wv/Wd [1024, 1024] f32, bd/gamma/beta [1024] f32.
    out = LayerNorm(ctx @ Wd.T + bd + hidden_states) with 16 heads, hd=64.

Sharding: pure data parallel. Core c handles batch b = c//4 and query rows
q in [ (c%4)*512, (c%4)*512+512 ).  Each core computes K/V for the full
sequence of its batch (4x redundant), attention + dense + LN for its own
512 rows.  No collectives.

On-device layout is transposed ("feature on partitions") end to end:
    xT   [1024, 2048]  (kv axis rolled so the core's own q rows come first)
    qT_h [64, 512], kT_h [64, 2048]  -> scores sT [kv, q] = kT.T-slices @ qT
    softmax: exp on ScalarE straight out of PSUM (scale=1/8, no max
    subtraction -- scores are O(3) by construction), mask applied as a
    bf16 multiply on VectorE, and the normalizer comes free from an extra
    all-ones column appended to V in the ctx matmul (row 64 of the
    transposed context = sum of masked probs).  Per head only two cheap
    drains run (unnormalized ctxT + sums row); normalization is batched at
    the end: one reciprocal over [16 heads, 512], a selector matmul that
    broadcasts each head pair's reciprocals to [128, 512], one multiply.
Projection work for head pairs 1..7 and v-group 1 is emitted as units
interleaved into the attention loop so the PE stream has no idle bubbles
(keeps the HAM clock gate at 2.4 GHz).  Dense: out[rows, 1024] with
lhsT = ctxT chunks, rhs = Wd.T chunks; add (x + bd) residual (bias folded
on host), LayerNorm on the free dim via bn_stats (gamma/beta passes are
compiled out when they are identity, which the spec's fills guarantee).
"""

import os
import sys
from contextlib import ExitStack

import numpy as np

for _p in ("/opt/trn_rl_repo",):
    if os.path.isdir(_p) and _p not in sys.path:
        sys.path.insert(0, _p)

import ml_dtypes  # noqa: E402

import concourse.bass as bass  # noqa: E402
import concourse.tile as tile  # noqa: E402
from concourse import bacc, mybir  # noqa: E402
from concourse.bass_utils import run_bass_kernel_spmd  # noqa: E402

BF16 = mybir.dt.bfloat16
F32 = mybir.dt.float32
NP_BF16 = ml_dtypes.bfloat16

B, S, H, NH = 2, 2048, 1024, 16
HD = H // NH  # 64
P = 128
NCORES = 8
SQ = S // 4  # 512 query rows per core
FC = H // P  # 8 feature chunks
KC = S // P  # 16 kv chunks
SCALE = 1.0 / float(np.sqrt(HD))
EPS = 1e-6

# Results of the last device run (for test harness introspection)
last_results = None


def _build_program(affine=True):
    nc = bacc.Bacc(
        "TRN2",
        target_bir_lowering=False,
        debug=False,
        enable_asserts=False,
        num_devices=NCORES,
    )

    # Per-core DRAM inputs
    d_xT = nc.dram_tensor("xT", [FC, P, S], BF16, kind="ExternalInput").ap()
    d_wq = nc.dram_tensor("wqT", [FC, P, H], BF16, kind="ExternalInput").ap()
    d_wk = nc.dram_tensor("wkT", [FC, P, H], BF16, kind="ExternalInput").ap()
    d_wv = nc.dram_tensor("wvT", [FC, P, H], BF16, kind="ExternalInput").ap()
    d_wd = nc.dram_tensor("wdT", [FC, P, H], BF16, kind="ExternalInput").ap()
    d_maskT = nc.dram_tensor("maskT", [KC, P, SQ], BF16, kind="ExternalInput").ap()
    d_xres = nc.dram_tensor("xres", [SQ // P, P, H], F32, kind="ExternalInput").ap()
    d_gamma = nc.dram_tensor("gamma", [H], F32, kind="ExternalInput").ap()
    d_beta = nc.dram_tensor("beta", [H], F32, kind="ExternalInput").ap()
    d_sel = nc.dram_tensor("sel", [NH, FC, P], F32, kind="ExternalInput").ap()
    d_out = nc.dram_tensor("out", [SQ // P, P, H], F32, kind="ExternalOutput").ap()

    with tile.TileContext(nc, trace_sim=False) as tc:
        _program(tc, d_xT, d_wq, d_wk, d_wv, d_wd, d_maskT, d_xres, d_gamma,
                 d_beta, d_sel, d_out, affine)

    nc.compile()
    return nc


def _bcast_ap(src_1d, parts):
    """AP that replicates a [n] DRAM vector across `parts` partitions."""
    return bass.AP(
        tensor=src_1d.tensor,
        offset=src_1d.offset,
        ap=[[0, parts]] + list(src_1d.ap),
    )


def _program(ctx_or_tc, *args):
    with ExitStack() as ctx:
        _program_inner(ctx, ctx_or_tc, *args)


def _program_inner(ctx, tc, d_xT, d_wq, d_wk, d_wv, d_wd, d_maskT, d_xres,
                   d_gamma, d_beta, d_sel, d_out, affine):
    from collections import deque
    nc = tc.nc

    # ---------------- pools ----------------
    persist = ctx.enter_context(tc.tile_pool(name="persist", bufs=1))
    ps_mm = ctx.enter_context(tc.tile_pool(name="ps_mm", bufs=2, space="PSUM"))
    ps_s = ctx.enter_context(tc.tile_pool(name="ps_s", bufs=2, space="PSUM"))
    ps_c = ctx.enter_context(tc.tile_pool(name="ps_c", bufs=2, space="PSUM"))

    # ---------------- persistent tiles (split per head-pair / v-group so
    # interleaved projection writes never collide with attention reads) ----
    kT_hp = [persist.tile([P, S], BF16, name=f"kT{hp}") for hp in range(FC)]
    qT_hp = [persist.tile([P, SQ], BF16, name=f"qT{hp}") for hp in range(FC)]
    v_g = [persist.tile([P, KC, 8, HD + 1], BF16, name=f"v{g}") for g in range(2)]
    ctxT_sb = persist.tile([P, FC, SQ], BF16)   # normalized in place per pair
    maskT_sb = persist.tile([P, KC, SQ], BF16)
    sums16 = persist.tile([NH, SQ], F32)

    nc.gpsimd.dma_start(out=maskT_sb, in_=d_maskT.rearrange("c p n -> p c n"))
    for g in range(2):
        nc.vector.memset(v_g[g][:, :, :, HD : HD + 1], 1.0)

    work = ctx.enter_context(tc.tile_pool(name="work", bufs=3))

    # ---------------- projections (emitted as units; the tail of them is
    # interleaved into the attention emission to fill PE gaps) ----------------
    from contextlib import ExitStack as _ES
    proj_ctx = _ES()
    pool_xt = proj_ctx.enter_context(tc.tile_pool(name="proj_xt", bufs=1))
    xT_sb = pool_xt.tile([P, FC, S], BF16)
    pool_w = proj_ctx.enter_context(tc.tile_pool(name="proj_w", bufs=1))
    wq_sb = pool_w.tile([P, FC, H], BF16)
    wk_sb = pool_w.tile([P, FC, H], BF16)
    wv_sb = pool_w.tile([P, FC, H], BF16)
    # interleaved across two HW queues so the first prefix matmuls can
    # start after ~2 small transfers instead of after one big serial load
    eng = [nc.sync, nc.scalar]
    nc.sync.dma_start(out=xT_sb[:, 0, :], in_=d_xT[0])
    nc.scalar.dma_start(out=xT_sb[:, 1, :], in_=d_xT[1])
    for half in range(2):
        cs = slice(half * 4, half * 4 + 4)
        eng[half].dma_start(out=wq_sb[:, cs, :],
                            in_=d_wq[cs].rearrange("c p n -> p c n"))
    for c in range(2, FC):
        eng[c % 2].dma_start(out=xT_sb[:, c, :], in_=d_xT[c])
    for half in range(2):
        cs = slice(half * 4, half * 4 + 4)
        eng[half].dma_start(out=wk_sb[:, cs, :],
                            in_=d_wk[cs].rearrange("c p n -> p c n"))
    nc.gpsimd.dma_start(out=wv_sb, in_=d_wv.rearrange("c p n -> p c n"))

    def unit_q(hp):
        qps = ps_mm.tile([P, SQ], F32, name="qps", tag="mm")
        for c in range(FC):
            nc.tensor.matmul(qps, lhsT=wq_sb[:, c, hp * P : (hp + 1) * P],
                             rhs=xT_sb[:, c, 0:SQ],
                             start=(c == 0), stop=(c == FC - 1))
        nc.vector.tensor_copy(qT_hp[hp], qps)

    def unit_k(hp, n):
        kps = ps_mm.tile([P, 512], F32, name="kps", tag="mm")
        for c in range(FC):
            nc.tensor.matmul(kps, lhsT=wk_sb[:, c, hp * P : (hp + 1) * P],
                             rhs=xT_sb[:, c, n * 512 : (n + 1) * 512],
                             start=(c == 0), stop=(c == FC - 1))
        nc.vector.tensor_copy(kT_hp[hp][:, n * 512 : (n + 1) * 512], kps)

    def unit_v(g, t):
        vps = ps_mm.tile([P, 512], F32, name="vps", tag="mm")
        for c in range(FC):
            nc.tensor.matmul(vps, lhsT=xT_sb[:, c, t * P : (t + 1) * P],
                             rhs=wv_sb[:, c, g * 512 : (g + 1) * 512],
                             start=(c == 0), stop=(c == FC - 1))
        nc.vector.tensor_copy(v_g[g][:, t, :, 0:HD],
                              vps.rearrange("p (h d) -> p h d", d=HD))

    # prefix: everything attention heads 0/1 need
    unit_q(0)
    for n in range(S // 512):
        unit_k(0, n)
    for t in range(KC):
        unit_v(0, t)

    units = deque()
    vg1 = deque((1, t) for t in range(KC))
    for hp in range(1, FC):
        units.append(("q", hp, 0))
        for n in range(S // 512):
            units.append(("k", hp, n))
        for _ in range(3):
            if vg1:
                units.append(("v", *vg1.popleft()))
    while vg1:
        units.append(("v", *vg1.popleft()))

    def emit_unit():
        if not units:
            return
        kind, a, b = units.popleft()
        if kind == "q":
            unit_q(a)
        elif kind == "k":
            unit_k(a, b)
        else:
            unit_v(a, b)

    # ---------------- attention (projection tail interleaved) ----------------
    CG = 2
    late = {}
    for h in range(NH):
        hp, hr = h // 2, (h % 2) * HD
        if h == FC:
            # all projection units have been emitted; free their SBUF and
            # bring in the dense/LN-phase tiles
            assert not units
            proj_ctx.close()
            late_pool = ctx.enter_context(tc.tile_pool(name="late", bufs=1))
            late["wd"] = late_pool.tile([P, FC, H], BF16, name="wd_sb")
            late["rec"] = late_pool.tile([NH, SQ], F32, name="rec_all")
            late["sel"] = late_pool.tile([NH, FC, P], F32, name="sel")
            nc.gpsimd.dma_start(out=late["sel"], in_=d_sel)
            nc.gpsimd.dma_start(out=late["wd"], in_=d_wd.rearrange("c p n -> p c n"))
        ctx_ps = ps_c.tile([P, SQ], F32, name="ctx_ps")
        for tg in range(KC // CG):
            s_ps = ps_s.tile([P, CG, SQ], F32, name="s_ps")
            for j in range(CG):
                t = tg * CG + j
                nc.tensor.matmul(
                    s_ps[:, j, :],
                    lhsT=kT_hp[hp][hr : hr + HD, t * P : (t + 1) * P],
                    rhs=qT_hp[hp][hr : hr + HD, :],
                    start=True, stop=True,
                )
            emit_unit()
            emit_unit()
            eT = work.tile([P, CG, SQ], BF16, name="eT")
            nc.scalar.activation(eT, s_ps, mybir.ActivationFunctionType.Exp,
                                 scale=SCALE)
            pT = work.tile([P, CG, SQ], BF16, name="pT")
            nc.vector.tensor_mul(pT, eT, maskT_sb[:, tg * CG : (tg + 1) * CG, :])
            for j in range(CG):
                t = tg * CG + j
                nc.tensor.matmul(
                    ctx_ps[0 : HD + 1, :],
                    lhsT=v_g[h // 8][:, t, h % 8, :],
                    rhs=pT[:, j, :],
                    start=(t == 0), stop=(t == KC - 1),
                )
        # cheap drains; division happens per head-pair as soon as both done
        nc.vector.tensor_copy(ctxT_sb[hr : hr + HD, hp, :], ctx_ps[0:HD, :])
        stmp = work.tile([1, SQ], F32, name="stmp")
        nc.vector.tensor_copy(stmp, ctx_ps[HD : HD + 1, :])
        # DMA sidesteps the partition-base restriction on compute engines
        nc.sync.dma_start(out=sums16[h : h + 1, :], in_=stmp)
    # batched normalization: one reciprocal over all heads, then per-head-pair
    # PE broadcast (selector matmul stacks both heads' recips) + one mul
    wd_sb = late["wd"]
    rec_all = late["rec"]
    nc.vector.reciprocal(rec_all, sums16)
    for cc in range(FC):
        bc_ps = ps_mm.tile([P, SQ], F32, name="bc_ps", tag="mm")
        nc.tensor.matmul(bc_ps, lhsT=late["sel"][:, cc, :], rhs=rec_all,
                         start=True, stop=True)
        nc.vector.tensor_mul(ctxT_sb[:, cc, :], ctxT_sb[:, cc, :], bc_ps)

    # ---------------- phase 3: dense + residual + LayerNorm ----------------
    ln_pool = ctx.enter_context(tc.tile_pool(name="ln", bufs=2))
    gb_pool = ctx.enter_context(tc.tile_pool(name="gb", bufs=1))
    eps_t = gb_pool.tile([P, 1], F32)
    nc.vector.memset(eps_t, EPS)
    if affine:
        gamma_bc = gb_pool.tile([P, H], F32)
        beta_bc = gb_pool.tile([P, H], F32)
        nc.sync.dma_start(out=gamma_bc, in_=_bcast_ap(d_gamma, P))
        nc.sync.dma_start(out=beta_bc, in_=_bcast_ap(d_beta, P))

    for r in range(SQ // P):
        pre = ln_pool.tile([P, H], F32, name="pre")
        xres_t = ln_pool.tile([P, H], F32, name="xres_t")
        nc.sync.dma_start(out=xres_t, in_=d_xres[r])
        for nh2 in range(H // 512):
            dps = ps_mm.tile([P, 512], F32, name="dps", tag="mm")
            for cc in range(FC):
                nc.tensor.matmul(
                    dps,
                    lhsT=ctxT_sb[:, cc, r * P : (r + 1) * P],
                    rhs=wd_sb[:, cc, nh2 * 512 : (nh2 + 1) * 512],
                    start=(cc == 0),
                    stop=(cc == FC - 1),
                )
            nc.vector.tensor_add(pre[:, nh2 * 512 : (nh2 + 1) * 512], dps,
                                 xres_t[:, nh2 * 512 : (nh2 + 1) * 512])

        # LayerNorm over free dim (1024) via bn_stats on two 512 subgroups
        stats = ln_pool.tile([P, 2, 6], F32, name="stats")
        nc.vector.bn_stats(stats[:, 0, :], pre[:, 0:512])
        nc.vector.bn_stats(stats[:, 1, :], pre[:, 512:1024])
        mv = ln_pool.tile([P, 2], F32, name="mv")
        nc.vector.bn_aggr(mv, stats)
        std = ln_pool.tile([P, 1], F32, name="std")
        nc.scalar.activation(std, mv[:, 1:2], mybir.ActivationFunctionType.Sqrt,
                             bias=eps_t)
        rstd = ln_pool.tile([P, 1], F32, name="rstd")
        nc.vector.reciprocal(rstd, std)
        outv = ln_pool.tile([P, H], F32, name="outv")
        nc.vector.tensor_scalar(outv, pre, mv[:, 0:1], rstd,
                                mybir.AluOpType.subtract, mybir.AluOpType.mult)
        if affine:
            nc.vector.tensor_mul(outv, outv, gamma_bc)
            nc.vector.tensor_add(outv, outv, beta_bc)
        nc.sync.dma_start(out=d_out[r], in_=outv)


_nc_cache = {}


def _get_nc(affine):
    if affine not in _nc_cache:
        _nc_cache[affine] = _build_program(affine)
    return _nc_cache[affine]


def kernel(hidden_states, mask, Wq, Wk, Wv, Wd, bd, gamma, beta):
    global last_results
    hidden_states = np.asarray(hidden_states, dtype=np.float32)
    mask = np.asarray(mask)
    Wq = np.asarray(Wq, dtype=np.float32)
    Wk = np.asarray(Wk, dtype=np.float32)
    Wv = np.asarray(Wv, dtype=np.float32)
    Wd = np.asarray(Wd, dtype=np.float32)
    bd = np.asarray(bd, dtype=np.float32)
    gamma = np.asarray(gamma, dtype=np.float32)
    beta = np.asarray(beta, dtype=np.float32)

    affine = bool(np.any(gamma != 1.0) or np.any(beta != 0.0))
    nc = _get_nc(affine)

    sel_np = np.zeros((NH, FC, P), dtype=np.float32)
    for cc in range(FC):
        sel_np[2 * cc, cc, 0:HD] = 1.0
        sel_np[2 * cc + 1, cc, HD:P] = 1.0

    wqT = np.ascontiguousarray(Wq.T).astype(NP_BF16).reshape(FC, P, H)
    wkT = np.ascontiguousarray(Wk.T).astype(NP_BF16).reshape(FC, P, H)
    wvT = np.ascontiguousarray(Wv.T).astype(NP_BF16).reshape(FC, P, H)
    wdT = np.ascontiguousarray(Wd.T).astype(NP_BF16).reshape(FC, P, H)

    in_maps = []
    for c in range(NCORES):
        b, qi = c // 4, c % 4
        qs = qi * SQ
        # roll the kv axis so this core's own query rows are columns 0..SQ
        xT = np.roll(hidden_states[b].T, -qs, axis=1)
        xT = np.ascontiguousarray(xT).astype(NP_BF16).reshape(FC, P, S)
        maskT = np.roll(mask[b].T, -qs, axis=0)[:, qs : qs + SQ]
        maskT = np.ascontiguousarray(maskT).astype(NP_BF16).reshape(KC, P, SQ)
        xres = (hidden_states[b, qs : qs + SQ] + bd[None, :]).astype(np.float32)
        in_maps.append({
            "xT": xT,
            "wqT": wqT,
            "wkT": wkT,
            "wvT": wvT,
            "wdT": wdT,
            "maskT": maskT,
            "xres": np.ascontiguousarray(xres.reshape(SQ // P, P, H)),
            "gamma": gamma,
            "beta": beta,
            "sel": sel_np,
        })

    trace = os.environ.get("BASS_KERNEL_TRACE", "0") == "1"
    res = run_bass_kernel_spmd(
        nc, in_maps, core_ids=list(range(NCORES)), trace=trace
    )
    last_results = res

    out = np.empty((B, S, H), dtype=np.float32)
    for c in range(NCORES):
        b, qi = c // 4, c % 4
        out[b, qi * SQ : (qi + 1) * SQ] = res.results[c]["out"].reshape(SQ, H)
    return out



# revision 9
# speedup vs baseline: 1.3761x; 1.3761x over previous
"""
Multi-head attention + residual + LayerNorm Trainium2 kernel (8 NeuronCores).

Problem (hardcoded shapes):
    hidden_states [2, 2048, 1024] f32, mask [2, 2048, 2048] int32,
    Wq/Wk/Wv/Wd [1024, 1024] f32, bd/gamma/beta [1024] f32.
    out = LayerNorm(ctx @ Wd.T + bd + hidden_states) with 16 heads, hd=64.

Sharding: pure data parallel. Core c handles batch b = c//4 and query rows
q in [ (c%4)*512, (c%4)*512+512 ).  Each core computes K/V for the full
sequence of its batch (4x redundant), attention + dense + LN for its own
512 rows.  No collectives.

Key engine-balance ideas vs the naive schedule:
  * Projections (Q/K/V) and dense run as fp8e4m3 DoubleRow matmuls (2 fp8
    weights per PE cell, ~1.44x).  Weights are scaled x64 on host so their
    ~0.02 magnitudes sit in fp8's normal range; the scale is folded into
    the softmax exp scale (q,k), the V ones-row (v) and a scale-invariant
    LayerNorm with eps*64^2 (dense+residual).
  * Projection PSUM->SBUF drains run on the otherwise idle GpSimd engine,
    freeing DVE for the mask multiplies.
  * Projection units are emitted interleaved into the attention stream on
    a need-by-step basis plus an even spreading quota, so the PE never
    idles while Act (exp) runs and vice versa.
  * Startup: x arrives as fp8 chunk-contiguous (2KB lines) over all four
    HW DGE queues; weight strips arrive per-head-pair so the first
    projection unit can start after ~1MB of traffic, not ~9MB.
"""

import os
import sys
from collections import deque
from contextlib import ExitStack

import numpy as np

for _p in ("/opt/trn_rl_repo",):
    if os.path.isdir(_p) and _p not in sys.path:
        sys.path.insert(0, _p)

import ml_dtypes  # noqa: E402

import concourse.bass as bass  # noqa: E402
import concourse.tile as tile  # noqa: E402
from concourse import bacc, mybir  # noqa: E402
from concourse.bass_utils import run_bass_kernel_spmd  # noqa: E402

BF16 = mybir.dt.bfloat16
F32 = mybir.dt.float32
FP8 = mybir.dt.float8e4
NP_BF16 = ml_dtypes.bfloat16
NP_FP8 = ml_dtypes.float8_e4m3

B, S, H, NH = 2, 2048, 1024, 16
HD = H // NH  # 64
P = 128
NCORES = 8
SQ = S // 4  # 512 query rows per core
FC = H // P  # 8 feature chunks
KC = S // P  # 16 kv chunks
NHP = FC  # 8 head pairs
WS = 64.0  # host-side weight scale (power of 2, keeps fp8 normal)
SCALE = 1.0 / (float(np.sqrt(HD)) * WS * WS)  # exp scale absorbs q,k scales
EPS = 1e-6 * WS * WS  # pre-LN tensor is x64 -> var x64^2; LN is scale-inv
CG = 2  # kv chunks per attention step
NSTEP = KC // CG  # 8 steps per head

DR = mybir.MatmulPerfMode.DoubleRow

# Results of the last device run (for test harness introspection)
last_results = None


def _build_program(affine=True):
    nc = bacc.Bacc(
        "TRN2",
        target_bir_lowering=False,
        debug=False,
        enable_asserts=False,
        num_devices=NCORES,
    )

    # Per-core DRAM inputs.  Weights are fp8, pre-scaled by WS, packed for
    # strip-wise (per head-pair / per v-half) DMA with fat lines.
    d_xT = nc.dram_tensor("xT", [FC, P, S], FP8, kind="ExternalInput").ap()
    d_wq = nc.dram_tensor("wqT", [NHP, P, FC * P], FP8, kind="ExternalInput").ap()
    d_wk = nc.dram_tensor("wkT", [NHP, P, FC * P], FP8, kind="ExternalInput").ap()
    d_wv = nc.dram_tensor("wvT", [2, P, FC * 512], FP8, kind="ExternalInput").ap()
    d_wd = nc.dram_tensor("wdT", [FC, P, H], FP8, kind="ExternalInput").ap()
    d_maskT = nc.dram_tensor("maskT", [KC, P, SQ], BF16, kind="ExternalInput").ap()
    d_xres = nc.dram_tensor("xres", [SQ // P, P, H], F32, kind="ExternalInput").ap()
    d_gamma = nc.dram_tensor("gamma", [H], F32, kind="ExternalInput").ap()
    d_beta = nc.dram_tensor("beta", [H], F32, kind="ExternalInput").ap()
    d_sel = nc.dram_tensor("sel", [NH, FC, P], BF16, kind="ExternalInput").ap()
    d_out = nc.dram_tensor("out", [SQ // P, P, H], F32, kind="ExternalOutput").ap()

    with tile.TileContext(nc, trace_sim=False) as tc:
        _program(tc, d_xT, d_wq, d_wk, d_wv, d_wd, d_maskT, d_xres, d_gamma,
                 d_beta, d_sel, d_out, affine)

    nc.compile()
    return nc


def _bcast_ap(src_1d, parts):
    """AP that replicates a [n] DRAM vector across `parts` partitions."""
    return bass.AP(
        tensor=src_1d.tensor,
        offset=src_1d.offset,
        ap=[[0, parts]] + list(src_1d.ap),
    )


def _program(ctx_or_tc, *args):
    with ExitStack() as ctx:
        _program_inner(ctx, ctx_or_tc, *args)


def _program_inner(ctx, tc, d_xT, d_wq, d_wk, d_wv, d_wd, d_maskT, d_xres,
                   d_gamma, d_beta, d_sel, d_out, affine):
    nc = tc.nc

    # ---------------- pools ----------------
    persist = ctx.enter_context(tc.tile_pool(name="persist", bufs=1))
    ps_mm = ctx.enter_context(tc.tile_pool(name="ps_mm", bufs=2, space="PSUM"))
    ps_s = ctx.enter_context(tc.tile_pool(name="ps_s", bufs=2, space="PSUM"))
    ps_c = ctx.enter_context(tc.tile_pool(name="ps_c", bufs=2, space="PSUM"))

    # ---------------- persistent tiles ----------------
    xT_sb = persist.tile([P, FC, S], FP8, name="xT_sb")
    wq_sb = persist.tile([P, NHP, FC, P], FP8, name="wq_sb")
    wk_sb = persist.tile([P, NHP, FC, P], FP8, name="wk_sb")
    wv_sb = persist.tile([P, 2, FC, 512], FP8, name="wv_sb")
    wd_sb = persist.tile([P, FC, H], FP8, name="wd_sb")
    kT_hp = [persist.tile([P, S], BF16, name=f"kT{hp}") for hp in range(NHP)]
    qT_hp = [persist.tile([P, SQ], BF16, name=f"qT{hp}") for hp in range(NHP)]
    v_g = [persist.tile([P, KC, 8, HD + 1], BF16, name=f"v{g}") for g in range(2)]
    ctxT_sb = persist.tile([P, FC, SQ], BF16, name="ctxT_sb")
    ctxT_f8 = persist.tile([P, FC, SQ], FP8, name="ctxT_f8")
    maskT_sb = persist.tile([P, KC, SQ], BF16, name="maskT_sb")
    sums16 = persist.tile([NH, SQ], F32, name="sums16")
    rec_all = persist.tile([NH, SQ], BF16, name="rec_all")
    sel_sb = persist.tile([NH, FC, P], BF16, name="sel_sb")

    # ---------------- DMA prefix (critical-path ordered) ----------------
    # 4 HW DGE queues: sync, scalar, vector, tensor; SW queue: gpsimd.
    # First-needed: x chunks (all 8) + wq strip0 + wk strip0 + wv half0 +
    # mask chunks 0..1.
    nc.scalar.dma_start(out=wq_sb[:, 0], in_=d_wq[0].rearrange("p (c n) -> p c n", n=P))
    nc.sync.dma_start(out=wk_sb[:, 0], in_=d_wk[0].rearrange("p (c n) -> p c n", n=P))
    qs = [nc.sync, nc.scalar]
    for c in range(FC):
        qs[c % 2].dma_start(out=xT_sb[:, c, :], in_=d_xT[c])
    nc.sync.dma_start(out=wv_sb[:, 0], in_=d_wv[0].rearrange("p (c n) -> p c n", n=512))
    nc.scalar.dma_start(out=maskT_sb[:, 0:4, :],
                        in_=d_maskT[0:4].rearrange("c p n -> p c n"))
    # second wave (needed within the first two head-pairs)
    nc.scalar.dma_start(out=wq_sb[:, 1], in_=d_wq[1].rearrange("p (c n) -> p c n", n=P))
    nc.sync.dma_start(out=wk_sb[:, 1], in_=d_wk[1].rearrange("p (c n) -> p c n", n=P))
    nc.sync.dma_start(out=maskT_sb[:, 4:16, :],
                      in_=d_maskT[4:16].rearrange("c p n -> p c n"))
    nc.sync.dma_start(out=wv_sb[:, 1], in_=d_wv[1].rearrange("p (c n) -> p c n", n=512))
    for hp in range(2, NHP):
        nc.scalar.dma_start(out=wq_sb[:, hp],
                            in_=d_wq[hp].rearrange("p (c n) -> p c n", n=P))
        nc.sync.dma_start(out=wk_sb[:, hp],
                          in_=d_wk[hp].rearrange("p (c n) -> p c n", n=P))
    nc.sync.dma_start(out=sel_sb, in_=d_sel)
    nc.sync.dma_start(out=wd_sb, in_=d_wd.rearrange("c p n -> p c n"))
    for g in range(2):
        nc.gpsimd.memset(v_g[g][:, :, :, HD : HD + 1], WS)

    work = ctx.enter_context(tc.tile_pool(name="work", bufs=3))

    # ---------------- projection units (fp8 DoubleRow) ----------------
    def unit_q(hp):
        qps = ps_mm.tile([P, SQ], F32, name="qps", tag="mm")
        for c2 in range(FC // 2):
            nc.tensor.matmul(qps,
                             lhsT=wq_sb[:, hp, 2 * c2 : 2 * c2 + 2, :],
                             rhs=xT_sb[:, 2 * c2 : 2 * c2 + 2, 0:SQ],
                             start=(c2 == 0), stop=(c2 == FC // 2 - 1),
                             perf_mode=DR)
        nc.vector.tensor_copy(qT_hp[hp], qps)

    def unit_k(hp, n):
        kps = ps_mm.tile([P, 512], F32, name="kps", tag="mm")
        for c2 in range(FC // 2):
            nc.tensor.matmul(kps,
                             lhsT=wk_sb[:, hp, 2 * c2 : 2 * c2 + 2, :],
                             rhs=xT_sb[:, 2 * c2 : 2 * c2 + 2, n * 512 : (n + 1) * 512],
                             start=(c2 == 0), stop=(c2 == FC // 2 - 1),
                             perf_mode=DR)
        nc.vector.tensor_copy(kT_hp[hp][:, n * 512 : (n + 1) * 512], kps)

    def unit_v(g, t):
        vps = ps_mm.tile([P, 512], F32, name="vps", tag="mm")
        for c2 in range(FC // 2):
            nc.tensor.matmul(vps,
                             lhsT=xT_sb[:, 2 * c2 : 2 * c2 + 2, t * P : (t + 1) * P],
                             rhs=wv_sb[:, g, 2 * c2 : 2 * c2 + 2, :],
                             start=(c2 == 0), stop=(c2 == FC // 2 - 1),
                             perf_mode=DR)
        nc.vector.tensor_copy(v_g[g][:, t, :, 0:HD],
                              vps.rearrange("p (h d) -> p h d", d=HD))

    # ---------------- unit schedule ----------------
    # Prefix: minimum to unblock head 0 step 0.
    unit_q(0)
    unit_k(0, 0)
    unit_v(0, 0)
    unit_v(0, 1)

    done_q = {0}
    done_k = {(0, 0)}
    done_v = {(0, 0), (0, 1)}

    # Remaining units ordered by earliest need-by (head, step); v-group 1
    # spread through the hp1..3 region, k strips just-in-time.
    units = deque()
    for t in range(2, KC):  # v(0,*) needed across head 0's steps
        units.append(("v", 0, t))
        if t % 4 == 3 and t // 4 < 4 and (0, t // 4) not in done_k:
            units.append(("k", 0, t // 4))
    for n in range(1, 4):
        if ("k", 0, n) not in units:
            units.append(("k", 0, n))
    for hp in range(1, NHP):
        units.append(("q", hp, 0))
        for n in range(4):
            units.append(("k", hp, n))
        if hp in (1, 2, 3):  # v group 1: 16 units over 3 head pairs
            base = (hp - 1) * 5
            for t in range(base, min(base + 5, KC)):
                units.append(("v", 1, t))
    units.append(("v", 1, 15))

    def emit_unit():
        kind, a, b2 = units.popleft()
        if kind == "q":
            unit_q(a)
            done_q.add(a)
        elif kind == "k":
            unit_k(a, b2)
            done_k.add((a, b2))
        else:
            unit_v(a, b2)
            done_v.add((a, b2))

    def require(kind, a, b2):
        tgt = (kind, a, b2)
        if kind == "q" and a in done_q:
            return
        if kind == "k" and (a, b2) in done_k:
            return
        if kind == "v" and (a, b2) in done_v:
            return
        assert tgt in units, f"missing unit {tgt}"
        while True:
            nxt = units[0]
            emit_unit()
            if nxt == tgt:
                break

    total_units = 4 + len(units)
    total_steps = NH * NSTEP
    emitted = [4]

    # ---------------- attention (units interleaved) ----------------
    for h in range(NH):
        hp, hr, g = h // 2, (h % 2) * HD, h // 8
        require("q", hp, 0)
        ctx_ps = ps_c.tile([P, SQ], F32, name="ctx_ps")
        for tg in range(NSTEP):
            require("k", hp, tg * CG // 4)
            for j in range(CG):
                require("v", g, tg * CG + j)
            s_ps = ps_s.tile([P, CG, SQ], F32, name="s_ps")
            for j in range(CG):
                t = tg * CG + j
                nc.tensor.matmul(
                    s_ps[:, j, :],
                    lhsT=kT_hp[hp][hr : hr + HD, t * P : (t + 1) * P],
                    rhs=qT_hp[hp][hr : hr + HD, :],
                    start=True, stop=True,
                )
            # spreading quota: keep residual units flowing between steps
            step_no = h * NSTEP + tg + 1
            quota = 4 + (total_units - 4) * step_no // total_steps
            while units and emitted[0] < quota:
                emit_unit()
                emitted[0] += 1
            emitted[0] = 4 + (total_units - 4) - len(units)
            eT = work.tile([P, CG, SQ], BF16, name="eT")
            nc.scalar.activation(eT, s_ps, mybir.ActivationFunctionType.Exp,
                                 scale=SCALE)
            pT = work.tile([P, CG, SQ], BF16, name="pT")
            nc.vector.tensor_mul(pT, eT, maskT_sb[:, tg * CG : (tg + 1) * CG, :])
            for j in range(CG):
                t = tg * CG + j
                nc.tensor.matmul(
                    ctx_ps[0 : HD + 1, :],
                    lhsT=v_g[g][:, t, h % 8, :],
                    rhs=pT[:, j, :],
                    start=(t == 0), stop=(t == KC - 1),
                )
        # cheap drains; normalization batched at the end
        nc.vector.tensor_copy(ctxT_sb[hr : hr + HD, hp, :], ctx_ps[0:HD, :])
        stmp = work.tile([1, SQ], F32, name="stmp")
        nc.vector.tensor_copy(stmp, ctx_ps[HD : HD + 1, :])
        # DMA sidesteps the partition-base restriction on compute engines
        nc.sync.dma_start(out=sums16[h : h + 1, :], in_=stmp)
    assert not units

    # batched normalization: one reciprocal over all heads, then per-head-pair
    # PE broadcast (selector matmul stacks both heads' recips) + one mul.
    # ctxT is written back as fp8 for the DoubleRow dense.
    with nc.allow_low_precision(reason="bf16 reciprocal: 0.4% normalization err ok"):
        nc.vector.reciprocal(rec_all, sums16)
    for cc in range(FC):
        bc_ps = ps_mm.tile([P, SQ], F32, name="bc_ps", tag="mm")
        nc.tensor.matmul(bc_ps, lhsT=sel_sb[:, cc, :], rhs=rec_all,
                         start=True, stop=True)
        nc.vector.tensor_mul(ctxT_f8[:, cc, :], ctxT_sb[:, cc, :], bc_ps)

    # ---------------- phase 3: dense + residual + LayerNorm ----------------
    ln_pool = ctx.enter_context(tc.tile_pool(name="ln", bufs=2))
    gb_pool = ctx.enter_context(tc.tile_pool(name="gb", bufs=1))
    eps_t = gb_pool.tile([P, 1], F32)
    nc.vector.memset(eps_t, EPS)
    if affine:
        gamma_bc = gb_pool.tile([P, H], F32)
        beta_bc = gb_pool.tile([P, H], F32)
        nc.sync.dma_start(out=gamma_bc, in_=_bcast_ap(d_gamma, P))
        nc.sync.dma_start(out=beta_bc, in_=_bcast_ap(d_beta, P))

    for r in range(SQ // P):
        pre = ln_pool.tile([P, H], F32, name="pre")
        xres_t = ln_pool.tile([P, H], F32, name="xres_t")
        nc.sync.dma_start(out=xres_t, in_=d_xres[r])
        for nh2 in range(H // 512):
            dps = ps_mm.tile([P, 512], F32, name="dps", tag="mm")
            for c2 in range(FC // 2):
                nc.tensor.matmul(
                    dps,
                    lhsT=ctxT_f8[:, 2 * c2 : 2 * c2 + 2, r * P : (r + 1) * P],
                    rhs=wd_sb[:, 2 * c2 : 2 * c2 + 2, nh2 * 512 : (nh2 + 1) * 512],
                    start=(c2 == 0), stop=(c2 == FC // 2 - 1),
                    perf_mode=DR,
                )
            nc.vector.tensor_add(pre[:, nh2 * 512 : (nh2 + 1) * 512], dps,
                                 xres_t[:, nh2 * 512 : (nh2 + 1) * 512])

        # LayerNorm over free dim (1024) via bn_stats on two 512 subgroups.
        # pre is 64x the reference pre-LN tensor; LN is scale-invariant with
        # eps scaled by 64^2.
        stats = ln_pool.tile([P, 2, 6], F32, name="stats")
        nc.vector.bn_stats(stats[:, 0, :], pre[:, 0:512])
        nc.vector.bn_stats(stats[:, 1, :], pre[:, 512:1024])
        mv = ln_pool.tile([P, 2], F32, name="mv")
        nc.vector.bn_aggr(mv, stats)
        std = ln_pool.tile([P, 1], F32, name="std")
        nc.scalar.activation(std, mv[:, 1:2], mybir.ActivationFunctionType.Sqrt,
                             bias=eps_t)
        rstd = ln_pool.tile([P, 1], F32, name="rstd")
        nc.vector.reciprocal(rstd, std)
        outv = ln_pool.tile([P, H], F32, name="outv")
        nc.vector.tensor_scalar(outv, pre, mv[:, 0:1], rstd,
                                mybir.AluOpType.subtract, mybir.AluOpType.mult)
        if affine:
            nc.vector.tensor_mul(outv, outv, gamma_bc)
            nc.vector.tensor_add(outv, outv, beta_bc)
        nc.sync.dma_start(out=d_out[r], in_=outv)


_nc_cache = {}


def _get_nc(affine):
    if affine not in _nc_cache:
        _nc_cache[affine] = _build_program(affine)
    return _nc_cache[affine]


def kernel(hidden_states, mask, Wq, Wk, Wv, Wd, bd, gamma, beta):
    global last_results
    hidden_states = np.asarray(hidden_states, dtype=np.float32)
    mask = np.asarray(mask)
    Wq = np.asarray(Wq, dtype=np.float32)
    Wk = np.asarray(Wk, dtype=np.float32)
    Wv = np.asarray(Wv, dtype=np.float32)
    Wd = np.asarray(Wd, dtype=np.float32)
    bd = np.asarray(bd, dtype=np.float32)
    gamma = np.asarray(gamma, dtype=np.float32)
    beta = np.asarray(beta, dtype=np.float32)

    affine = bool(np.any(gamma != 1.0) or np.any(beta != 0.0))
    nc = _get_nc(affine)

    sel_np = np.zeros((NH, FC, P), dtype=np.float32)
    for cc in range(FC):
        sel_np[2 * cc, cc, 0:HD] = 1.0
        sel_np[2 * cc + 1, cc, HD:P] = 1.0
    sel_np = sel_np.astype(NP_BF16)

    # weight packing (scaled x64 into fp8 normal range)
    # wq/wk strips: [hp][p][c*128+n] = W.T[c*128+p, hp*128+n]
    wqT = (np.ascontiguousarray(Wq.T) * WS).astype(NP_FP8).reshape(FC, P, FC, P)
    wqT = np.ascontiguousarray(wqT.transpose(2, 1, 0, 3)).reshape(NHP, P, FC * P)
    wkT = (np.ascontiguousarray(Wk.T) * WS).astype(NP_FP8).reshape(FC, P, FC, P)
    wkT = np.ascontiguousarray(wkT.transpose(2, 1, 0, 3)).reshape(NHP, P, FC * P)
    # wv halves: [g][p][c*512+n] = Wv.T[c*128+p, g*512+n]
    wvT = (np.ascontiguousarray(Wv.T) * WS).astype(NP_FP8).reshape(FC, P, 2, 512)
    wvT = np.ascontiguousarray(wvT.transpose(2, 1, 0, 3)).reshape(2, P, FC * 512)
    wdT = (np.ascontiguousarray(Wd.T) * WS).astype(NP_FP8).reshape(FC, P, H)

    in_maps = []
    for c in range(NCORES):
        b, qi = c // 4, c % 4
        qs = qi * SQ
        # roll the kv axis so this core's own query rows are columns 0..SQ
        xT = np.roll(hidden_states[b].T, -qs, axis=1)
        xT = np.ascontiguousarray(xT).astype(NP_FP8).reshape(FC, P, S)
        maskT = np.roll(mask[b].T, -qs, axis=0)[:, qs : qs + SQ]
        maskT = np.ascontiguousarray(maskT).astype(NP_BF16).reshape(KC, P, SQ)
        xres = ((hidden_states[b, qs : qs + SQ] + bd[None, :]) * WS).astype(
            np.float32)
        in_maps.append({
            "xT": xT,
            "wqT": wqT,
            "wkT": wkT,
            "wvT": wvT,
            "wdT": wdT,
            "maskT": maskT,
            "xres": np.ascontiguousarray(xres.reshape(SQ // P, P, H)),
            "gamma": gamma,
            "beta": beta,
            "sel": sel_np,
        })

    trace = os.environ.get("BASS_KERNEL_TRACE", "0") == "1"
    res = run_bass_kernel_spmd(
        nc, in_maps, core_ids=list(range(NCORES)), trace=trace
    )
    last_results = res

    out = np.empty((B, S, H), dtype=np.float32)
    for c in range(NCORES):
        b, qi = c // 4, c % 4
        out[b, qi * SQ : (qi + 1) * SQ] = res.results[c]["out"].reshape(SQ, H)
    return out
